# revision 21
# baseline (speedup 1.0000x reference)
"""DynamicGCN edge-MLP message passing kernel for 8x trn2 NeuronCores.

Shapes (hardcoded): x [2, 512, 256] f32, base_adj [2, 512, 512] f32,
W1 [512, 256], b1 [256], W2 [256, 128], b2 [128], W3 [128, 1], b3 [1],
Wg [256, 256], bg [256].  Output [2, 512, 256] f32.

Sharding: core c handles batch b = c // 4 and query rows
i in [128*(c%4), 128*(c%4)+128).  Params replicated; all per-core
variation is input data (same NEFF on all 8 cores).

Math (per core, i-block of 128 rows):
  leftb[c,i]  = (x_i @ W1[:D])[c] + b1[c]          (PE prologue)
  rightT[c,j] = (x_j @ W1[D:])[c]                  (PE prologue)
  per i:
    hT[c,j]   = relu(rightT[c,j] + leftb[c,i])     (DVE dual-scalar op)
    h2p[k,j]  = sum_c hT[c,j] * (W2*|W3|)[c,k]     (PE, 2 accum matmuls)
    t[k,j]    = relu(h2p[k,j] + |W3_k| b2_k)       (ACT, psum->sbuf)
    edge[i,j] += sum_k sign(W3_k) t[k,j]           (PE matmul, shifted
                  sign-column stationary operand writes psum row i)
  edge[i,:]   = h2(i,j,:) @ W3  (exact)
  s = tanh(.5*edge + .5*b3); adj = .5*badj*(1+s) + eye = .5badj*s + bhae
  adjn = softmax(adj) rowwise  (exp + fused accum rowsum + reciprocal)
  out = (adjn @ x_b) @ Wg + bg  (PE transposes + 2 matmul chains)
"""

import ml_dtypes
import numpy as np

import concourse.bacc as bacc
import concourse.bass as bass
import concourse.mybir as mybir
import concourse.tile as tile
from concourse.bass_utils import run_bass_kernel_spmd

F32 = mybir.dt.float32
F32R = mybir.dt.float32r
BF16 = mybir.dt.bfloat16
AF = mybir.ActivationFunctionType
OP = mybir.AluOpType

P = 128      # partitions / i-block
N = 512      # nodes (j dim)
D = 256      # input dim
H = 256      # hidden (c dim, 2 partition tiles)
H2 = 128     # second hidden (k dim)
NCORES = 8
LAG = 4      # software-pipeline depth (in 2-row groups)
ACT_EVERY = 0   # route ht1 stage-A to ACT every Nth group (0 = never)


def _build_program():
    nc = bacc.Bacc("TRN2", target_bir_lowering=False, debug=False)

    def din(name, shape, dtype=F32):
        return nc.dram_tensor(name, list(shape), dtype, kind="ExternalInput").ap()

    xt = din("xt", (2, P, N))          # x_b.T d-tiles        [dt][d, j]
    xti = din("xti", (2, P, P))        # x_i-block.T d-tiles  [dt][d, i]
    xb = din("xb", (4, P, D))          # x_b j-tiles          [jt][j, d]
    w1a = din("w1a", (2, P, H))        # W1[:D] d-tiles       [dt][d, c]
    w1b = din("w1b", (2, P, H))        # W1[D:] d-tiles       [dt][d, c]
    w2s = din("w2s", (2, P, H2), BF16)  # (W2*|W3|) c-tiles   [ct][c, k]
    wg = din("wg", (2, P, D))          # Wg d-tiles           [dt][d, h]
    b1c = din("b1c", (2, P, 1))        # b1 per c-tile column
    b2sc = din("b2sc", (P, 1))         # |W3|*b2 column
    sgnsh = din("sgnsh", (P, 64), BF16)  # sign(W3) at col 32, else 0
    halfb3 = din("halfb3", (P, 1))     # 0.5*b3 column
    bha = din("bha", (P, N))           # 0.5*base_adj rows
    bhae = din("bhae", (P, N))         # 0.5*base_adj + eye rows
    ident = din("ident", (P, P))       # identity for PE transpose
    bgt = din("bgt", (P, D))           # bg broadcast
    out_d = nc.dram_tensor("out", [P, D], F32, kind="ExternalOutput").ap()

    with tile.TileContext(nc) as tc:
        with (
            tc.tile_pool(name="const", bufs=1) as const,
            tc.tile_pool(name="work", bufs=8) as work,
            tc.tile_pool(name="pedge", bufs=1, space="PSUM") as pedge,
        ):
            # ---- persistent SBUF loads ----
            xt_sb = const.tile([P, 2, N], F32)
            xti_sb = const.tile([P, 2, P], F32)
            xb_sb = const.tile([P, 4, D], F32)
            w1a_sb = const.tile([P, 2, H], F32)
            w1b_sb = const.tile([P, 2, H], F32)
            w2s_sb = const.tile([P, 2, H2], BF16)
            wg_sb = const.tile([P, 2, D], F32)
            b1c_sb = const.tile([P, 2], F32)
            b2sc_sb = const.tile([P, 1], F32)
            sgnsh_sb = const.tile([P, 64], BF16)
            halfb3_sb = const.tile([P, 1], F32)
            bha_sb = const.tile([P, N], F32)
            bhae_sb = const.tile([P, N], F32)
            ident_sb = const.tile([P, P], F32)
            bgt_sb = const.tile([P, D], F32)

            # critical-path loads on the sync queue; the rest on gpsimd's
            # queue so the prologue doesn't serialize behind one engine.
            for dt in range(2):
                nc.sync.dma_start(xt_sb[:, dt, :], xt[dt])
                nc.sync.dma_start(w1b_sb[:, dt, :], w1b[dt])
                nc.sync.dma_start(w2s_sb[:, dt, :], w2s[dt])
                nc.sync.dma_start(xti_sb[:, dt, :], xti[dt])
                nc.sync.dma_start(w1a_sb[:, dt, :], w1a[dt])
                nc.sync.dma_start(b1c_sb[:, dt : dt + 1], b1c[dt])
            nc.sync.dma_start(b2sc_sb[:], b2sc)
            nc.sync.dma_start(sgnsh_sb[:], sgnsh)
            for jt in range(4):
                nc.gpsimd.dma_start(xb_sb[:, jt, :], xb[jt])
            for dt in range(2):
                nc.gpsimd.dma_start(wg_sb[:, dt, :], wg[dt])
            nc.gpsimd.dma_start(halfb3_sb[:], halfb3)
            nc.gpsimd.dma_start(bha_sb[:], bha)
            nc.gpsimd.dma_start(bhae_sb[:], bhae)
            nc.gpsimd.dma_start(ident_sb[:], ident)
            nc.gpsimd.dma_start(bgt_sb[:], bgt)

            # Preload the exp/tanh/relu activation table set early so the
            # ~2.7us table DMA overlaps the input DMAs.
            warm = const.tile([P, 1], F32)
            nc.scalar.activation(warm[:], halfb3_sb[:], AF.Exp)

            edge_ps = pedge.tile([P, N], F32)

            with tc.tile_pool(name="ph2", bufs=3, space="PSUM") as ph2:
                # ---- prologue: rightT / leftb ----
                rightT_sb = const.tile([P, 2, N], BF16)
                leftb_sb = const.tile([P, 2, P], F32)
                for ct in range(2):
                    ps = ph2.tile([P, N], F32, tag="h2")
                    csl = bass.ts(ct, P)
                    for dt in range(2):
                        nc.tensor.matmul(
                            ps[:],
                            w1b_sb[:, dt, csl],
                            xt_sb[:, dt, :],
                            start=(dt == 0),
                            stop=(dt == 1),
                        )
                    nc.scalar.copy(rightT_sb[:, ct, :], ps[:])
                for ct in range(2):
                    ps = ph2.tile([P, N], F32, tag="h2")
                    csl = bass.ts(ct, P)
                    for dt in range(2):
                        nc.tensor.matmul(
                            ps[:, :P],
                            w1a_sb[:, dt, csl],
                            xti_sb[:, dt, :],
                            start=(dt == 0),
                            stop=(dt == 1),
                        )
                    nc.scalar.activation(
                        leftb_sb[:, ct, :], ps[:, :P], AF.Identity,
                        bias=b1c_sb[:, ct : ct + 1], scale=1.0,
                    )

                # ---- main loop over the 128 query rows, 2 rows per group ----
                NG = P // 2
                h2ps = {}
                for step in range(NG + LAG):
                    if step < NG:
                        g = step
                        hts = []
                        for u in range(2):
                            i = 2 * g + u
                            ht0 = work.tile([P, N], BF16, tag=f"ht0{u}")
                            ht1 = work.tile([P, N], BF16, tag=f"ht1{u}")
                            nc.vector.tensor_scalar(
                                ht0[:], rightT_sb[:, 0, :],
                                leftb_sb[:, 0, i : i + 1], 0.0,
                                op0=OP.add, op1=OP.max,
                            )
                            # route some stage-A work to ACT to balance engines
                            if ACT_EVERY and g % ACT_EVERY == 0 and u == 1:
                                nc.scalar.activation(
                                    ht1[:], rightT_sb[:, 1, :], AF.Relu,
                                    bias=leftb_sb[:, 1, i : i + 1], scale=1.0,
                                )
                            else:
                                nc.vector.tensor_scalar(
                                    ht1[:], rightT_sb[:, 1, :],
                                    leftb_sb[:, 1, i : i + 1], 0.0,
                                    op0=OP.add, op1=OP.max,
                                )
                            hts.append((ht0, ht1))
                        ps = ph2.tile([P, 2, N], F32, tag="h2")
                        for u in range(2):
                            nc.tensor.matmul(
                                ps[:, u, :], w2s_sb[:, 0, :], hts[u][0][:],
                                start=True, stop=False,
                            )
                        for u in range(2):
                            nc.tensor.matmul(
                                ps[:, u, :], w2s_sb[:, 1, :], hts[u][1][:],
                                start=False, stop=True,
                            )
                        h2ps[g] = ps
                    gj = step - LAG
                    if gj >= 0:
                        t_sb = work.tile([P, 2, N], BF16, tag="tt")
                        nc.scalar.activation(
                            t_sb[:], h2ps.pop(gj)[:], AF.Relu,
                            bias=b2sc_sb[:], scale=1.0,
                        )
                        for u in range(2):
                            j = 2 * gj + u
                            cg, r = divmod(j, 32)
                            nc.tensor.matmul(
                                edge_ps[32 * cg : 32 * cg + 32, :],
                                sgnsh_sb[:, 32 - r : 64 - r],
                                t_sb[:, u, :],
                                start=(r == 0),
                                stop=(r == 31),
                                tile_position=(0, 32 * cg),
                            )

            # ---- tail: sigmoid via tanh, softmax, aggregate, project ----
            with tc.tile_pool(name="ptail", bufs=2, space="PSUM") as ptail:
                s_sb = const.tile([P, N], F32)
                nc.scalar.activation(
                    s_sb[:], edge_ps[:], AF.Tanh, bias=halfb3_sb[:], scale=0.5
                )
                m1 = const.tile([P, N], F32)
                nc.vector.tensor_mul(m1[:], s_sb[:], bha_sb[:])
                m2 = const.tile([P, N], F32)
                nc.vector.tensor_add(m2[:], m1[:], bhae_sb[:])
                adjexp = const.tile([P, N], F32)
                rowsum = const.tile([P, 1], F32)
                nc.scalar.activation(adjexp[:], m2[:], AF.Exp, accum_out=rowsum[:])
                invs = const.tile([P, 1], F32)
                nc.vector.reciprocal(invs[:], rowsum[:])
                adjn = const.tile([P, N], F32)
                nc.vector.tensor_scalar_mul(adjn[:], adjexp[:], invs[:])

                adjnT = const.tile([P, 4, P], F32)
                for jt in range(4):
                    pt = ptail.tile([P, P], F32, tag="pt")
                    nc.tensor.transpose(
                        pt[:], adjn[:, bass.ts(jt, P)], ident_sb[:]
                    )
                    nc.any.tensor_copy(adjnT[:, jt, :], pt[:])

                aggT_sb = const.tile([P, 2, P], F32)
                for dh in range(2):
                    pa = ptail.tile([P, P], F32, tag="pa")
                    for jt in range(4):
                        nc.tensor.matmul(
                            pa[:],
                            xb_sb[:, jt, bass.ts(dh, P)],
                            adjnT[:, jt, :],
                            start=(jt == 0),
                            stop=(jt == 3),
                        )
                    nc.any.tensor_copy(aggT_sb[:, dh, :], pa[:])

                po = ptail.tile([P, D], F32, tag="po")
                for dh in range(2):
                    nc.tensor.matmul(
                        po[:], aggT_sb[:, dh, :], wg_sb[:, dh, :],
                        start=(dh == 0), stop=(dh == 1),
                    )
                out_sb = const.tile([P, D], F32)
                nc.vector.tensor_add(out_sb[:], po[:], bgt_sb[:])
                nc.sync.dma_start(out_d[:], out_sb[:])

    nc.compile()
    return nc


_NC = None


def _get_program():
    global _NC
    if _NC is None:
        _NC = _build_program()
    return _NC


def _core_inputs(x, base_adj, W1, b1, W2, b2, W3, b3, Wg, bg, core):
    b, blk = divmod(core, 4)
    i0 = blk * P
    f32 = np.float32

    xbf = np.ascontiguousarray(x[b], dtype=f32)               # [512, 256]
    xtf = np.ascontiguousarray(xbf.T)                         # [256, 512]
    w3 = np.asarray(W3, dtype=f32)[:, 0]                      # [128]

    bf16 = ml_dtypes.bfloat16
    sgnsh = np.zeros((P, 64), dtype=bf16)
    sgnsh[:, 32] = np.sign(w3).astype(bf16)

    bha = 0.5 * base_adj[b, i0 : i0 + P, :].astype(f32)
    eye = np.zeros((P, N), dtype=f32)
    eye[np.arange(P), i0 + np.arange(P)] = 1.0

    return {
        "xt": xtf.reshape(2, P, N),
        "xti": np.ascontiguousarray(xbf[i0 : i0 + P, :].T).reshape(2, P, P),
        "xb": xbf.reshape(4, P, D),
        "w1a": np.ascontiguousarray(W1[:D], dtype=f32).reshape(2, P, H),
        "w1b": np.ascontiguousarray(W1[D:], dtype=f32).reshape(2, P, H),
        "w2s": np.ascontiguousarray(
            W2.astype(f32) * np.abs(w3)[None, :]
        ).astype(bf16).reshape(2, P, H2),
        "wg": np.ascontiguousarray(Wg, dtype=f32).reshape(2, P, D),
        "b1c": np.asarray(b1, dtype=f32).reshape(2, P, 1),
        "b2sc": (np.abs(w3) * np.asarray(b2, dtype=f32)).reshape(P, 1),
        "sgnsh": sgnsh,
        "halfb3": np.full((P, 1), 0.5 * float(np.asarray(b3).reshape(-1)[0]), f32),
        "bha": np.ascontiguousarray(bha),
        "bhae": np.ascontiguousarray(bha + eye),
        "ident": np.eye(P, dtype=f32),
        "bgt": np.tile(np.asarray(bg, dtype=f32)[None, :], (P, 1)),
    }


def run(trace=False, **inputs):
    nc = _get_program()
    in_maps = [_core_inputs(core=c, **inputs) for c in range(NCORES)]
    res = run_bass_kernel_spmd(
        nc, in_maps, core_ids=list(range(NCORES)), trace=trace
    )
    out = np.empty((2, N, D), dtype=np.float32)
    for c in range(NCORES):
        b, blk = divmod(c, 4)
        out[b, blk * P : (blk + 1) * P, :] = res.results[c]["out"]
    return out, res


def kernel(**inputs):
    out, _ = run(**inputs)
    return out


# revision 23
# speedup vs baseline: 1.2080x; 1.2080x over previous
"""DynamicGCN edge-MLP message passing kernel for 8x trn2 NeuronCores.

Shapes (hardcoded): x [2, 512, 256] f32, base_adj [2, 512, 512] f32,
W1 [512, 256], b1 [256], W2 [256, 128], b2 [128], W3 [128, 1], b3 [1],
Wg [256, 256], bg [256].  Output [2, 512, 256] f32.

Sharding: core c handles batch b = c // 4 and query rows
i in [128*(c%4), 128*(c%4)+128).  Params replicated; all per-core
variation is input data (same NEFF on all 8 cores).

Per core (i-block of 128 query rows, j = all 512 nodes):
  leftb[c,i]  = (x_i @ W1[:D])[c] + b1[c]          (PE prologue)
  rightT[c,j] = (x_j @ W1[D:])[c]                  (PE prologue, bf16)
  per i:
    hT[c,j]   = relu(rightT[c,j] + leftb[c,i])     (DVE dual-scalar op, bf16)
    h2p[k,j]  = sum_c hT[c,j] * (W2*|W3|)[c,k]     (PE, 2 accum matmuls)
    t[k,j]    = relu(h2p[k,j] + |W3_k| b2_k)       (ACT, psum->sbuf, bf16)
    edge[i,j] += sum_k sign(W3_k) t[k,j]           (PE matmul; shifted
                  sign-column stationary operand accumulates psum row i
                  via 32-wide output col-groups)
  edge[i,:] == h2(i,j,:) @ W3  exactly (|W3| folded into W2/b2, sign into
  the reduction weights; relu is positively homogeneous)
  s = tanh(.5*edge + .5*b3)  => sigmoid without a table switch
  adj = .5*badj*(1+s) + eye = .5*badj*s + (.5*badj + eye)
  adjn = softmax(adj) rowwise (exp with fused accum row-sum, reciprocal)
  out = (adjn @ x_b) @ Wg + bg  (PE transposes + 2 matmul chains)

Inputs are packed host-side into 3 DRAM tensors (one bf16 critical, one
f32 critical, one f32 late) so the prologue is 3 large DMAs instead of
26 small ones.
"""

import ml_dtypes
import numpy as np

import concourse.bacc as bacc
import concourse.bass as bass
import concourse.mybir as mybir
import concourse.tile as tile
from concourse.bass_utils import run_bass_kernel_spmd

F32 = mybir.dt.float32
BF16 = mybir.dt.bfloat16
AF = mybir.ActivationFunctionType
OP = mybir.AluOpType

P = 128      # partitions / i-block
N = 512      # nodes (j dim)
D = 256      # input dim
H = 256      # hidden (c dim, 2 partition tiles)
H2 = 128     # second hidden (k dim)
NCORES = 8
LAG = 4      # software-pipeline depth (in 2-row groups)

# --- packed-input layouts (free-dim element offsets) ---
# bigh (bf16): xt[2]@512, w1b[2]@256(c x2 tiles of 256? see below), w2s[2]@128, sgnsh@64
#   xt tiles:   [0:512), [512:1024)         (d-tile rows x j)
#   w1b tiles:  [1024:1280), [1280:1536)    (d-tile rows x c)  (256 each)
#   w2s tiles:  [1536:1664), [1664:1792)    (c-tile rows x k)
#   sgnsh:      [1792:1856)
BH_XT = 0
BH_W1B = 1024
BH_W2S = 1536
BH_SGN = 1792
BH_TOT = 1856
# bigf (f32, critical): xti[2]@128, w1a[2]@256, b1c@2, b2sc@1, halfb3@1
BF_XTI = 0
BF_W1A = 256
BF_B1C = 768
BF_B2SC = 770
BF_HB3 = 771
BF_TOT = 772
# bigl (f32, late): xb[4]@256, wg[2]@256, bha@512, bhae@512, ident@128, bgt@256
BL_XB = 0
BL_WG = 1024
BL_BHA = 1536
BL_BHAE = 2048
BL_ID = 2560
BL_BGT = 2688
BL_TOT = 2944


def _build_program():
    nc = bacc.Bacc("TRN2", target_bir_lowering=False, debug=False)

    bigh = nc.dram_tensor("bigh", [P, BH_TOT], BF16, kind="ExternalInput").ap()
    bigf = nc.dram_tensor("bigf", [P, BF_TOT], F32, kind="ExternalInput").ap()
    bigl = nc.dram_tensor("bigl", [P, BL_TOT], F32, kind="ExternalInput").ap()
    out_d = nc.dram_tensor("out", [P, D], F32, kind="ExternalOutput").ap()

    with tile.TileContext(nc) as tc:
        with (
            tc.tile_pool(name="const", bufs=1) as const,
            tc.tile_pool(name="work", bufs=8) as work,
            tc.tile_pool(name="pedge", bufs=1, space="PSUM") as pedge,
        ):
            bh = const.tile([P, BH_TOT], BF16)
            bf = const.tile([P, BF_TOT], F32)
            bl = const.tile([P, BL_TOT], F32)
            nc.sync.dma_start(bh[:], bigh)
            nc.sync.dma_start(bf[:], bigf)
            nc.gpsimd.dma_start(bl[:], bigl)

            def xt_sb(dt):
                return bh[:, BH_XT + 512 * dt : BH_XT + 512 * (dt + 1)]

            def w1b_sb(dt, csl):
                base = BH_W1B + 256 * dt
                return bh[:, base + csl * 128 : base + csl * 128 + 128]

            def w2s_sb(ct):
                return bh[:, BH_W2S + 128 * ct : BH_W2S + 128 * (ct + 1)]

            sgnsh_sb = bh[:, BH_SGN : BH_SGN + 64]

            def xti_sb(dt):
                return bf[:, BF_XTI + 128 * dt : BF_XTI + 128 * (dt + 1)]

            def w1a_sb(dt, csl):
                base = BF_W1A + 256 * dt
                return bf[:, base + csl * 128 : base + csl * 128 + 128]

            b1c_sb = bf[:, BF_B1C : BF_B1C + 2]
            b2sc_sb = bf[:, BF_B2SC : BF_B2SC + 1]
            halfb3_sb = bf[:, BF_HB3 : BF_HB3 + 1]

            def xb_sb(jt):
                return bl[:, BL_XB + 256 * jt : BL_XB + 256 * (jt + 1)]

            def wg_sb(dt):
                return bl[:, BL_WG + 256 * dt : BL_WG + 256 * (dt + 1)]

            bha_sb = bl[:, BL_BHA : BL_BHA + 512]
            bhae_sb = bl[:, BL_BHAE : BL_BHAE + 512]
            ident_sb = bl[:, BL_ID : BL_ID + 128]
            bgt_sb = bl[:, BL_BGT : BL_BGT + 256]

            # Preload the exp/tanh/relu activation table set early so the
            # ~2.7us table DMA overlaps the input DMAs.
            warm = const.tile([P, 1], F32)
            nc.vector.memset(warm[:], 0.0)
            nc.scalar.activation(warm[:], warm[:], AF.Exp)

            edge_ps = pedge.tile([P, N], F32)

            with tc.tile_pool(name="ph2", bufs=3, space="PSUM") as ph2:
                # ---- prologue: rightT / leftb ----
                rightT_sb = const.tile([P, 2, N], BF16)
                leftb_sb = const.tile([P, 2, P], F32)
                for ct in range(2):
                    ps = ph2.tile([P, 2, N], F32, tag="h2")
                    for dt in range(2):
                        nc.tensor.matmul(
                            ps[:, 0, :],
                            w1b_sb(dt, ct),
                            xt_sb(dt),
                            start=(dt == 0),
                            stop=(dt == 1),
                        )
                    nc.scalar.copy(rightT_sb[:, ct, :], ps[:, 0, :])
                for ct in range(2):
                    ps = ph2.tile([P, 2, N], F32, tag="h2")
                    for dt in range(2):
                        nc.tensor.matmul(
                            ps[:, 0, :P],
                            w1a_sb(dt, ct),
                            xti_sb(dt),
                            start=(dt == 0),
                            stop=(dt == 1),
                        )
                    nc.scalar.activation(
                        leftb_sb[:, ct, :], ps[:, 0, :P], AF.Identity,
                        bias=b1c_sb[:, ct : ct + 1], scale=1.0,
                    )

                # ---- main loop over the 128 query rows, 2 rows per group ----
                NG = P // 2
                h2ps = {}
                for step in range(NG + LAG):
                    if step < NG:
                        g = step
                        hts = []
                        for u in range(2):
                            i = 2 * g + u
                            ht0 = work.tile([P, N], BF16, tag=f"ht0{u}")
                            ht1 = work.tile([P, N], BF16, tag=f"ht1{u}")
                            nc.vector.tensor_scalar(
                                ht0[:], rightT_sb[:, 0, :],
                                leftb_sb[:, 0, i : i + 1], 0.0,
                                op0=OP.add, op1=OP.max,
                            )
                            nc.vector.tensor_scalar(
                                ht1[:], rightT_sb[:, 1, :],
                                leftb_sb[:, 1, i : i + 1], 0.0,
                                op0=OP.add, op1=OP.max,
                            )
                            hts.append((ht0, ht1))
                        ps = ph2.tile([P, 2, N], F32, tag="h2")
                        for u in range(2):
                            nc.tensor.matmul(
                                ps[:, u, :], w2s_sb(0), hts[u][0][:],
                                start=True, stop=False,
                            )
                        for u in range(2):
                            nc.tensor.matmul(
                                ps[:, u, :], w2s_sb(1), hts[u][1][:],
                                start=False, stop=True,
                            )
                        h2ps[g] = ps
                    gj = step - LAG
                    if gj >= 0:
                        t_sb = work.tile([P, 2, N], BF16, tag="tt")
                        nc.scalar.activation(
                            t_sb[:], h2ps.pop(gj)[:], AF.Relu,
                            bias=b2sc_sb[:], scale=1.0,
                        )
                        for u in range(2):
                            j = 2 * gj + u
                            cg, r = divmod(j, 32)
                            nc.tensor.matmul(
                                edge_ps[32 * cg : 32 * cg + 32, :],
                                sgnsh_sb[:, 32 - r : 64 - r],
                                t_sb[:, u, :],
                                start=(r == 0),
                                stop=(r == 31),
                                tile_position=(0, 32 * cg),
                            )

            # ---- tail: sigmoid via tanh, softmax, aggregate, project ----
            with tc.tile_pool(name="ptail", bufs=2, space="PSUM") as ptail:
                s_sb = const.tile([P, N], F32)
                nc.scalar.activation(
                    s_sb[:], edge_ps[:], AF.Tanh, bias=halfb3_sb, scale=0.5
                )
                m1 = const.tile([P, N], F32)
                nc.vector.tensor_mul(m1[:], s_sb[:], bha_sb)
                m2 = const.tile([P, N], F32)
                nc.vector.tensor_add(m2[:], m1[:], bhae_sb)
                adjexp = const.tile([P, N], F32)
                rowsum = const.tile([P, 1], F32)
                nc.scalar.activation(adjexp[:], m2[:], AF.Exp, accum_out=rowsum[:])
                invs = const.tile([P, 1], F32)
                nc.vector.reciprocal(invs[:], rowsum[:])
                adjn = const.tile([P, N], F32)
                nc.vector.tensor_scalar_mul(adjn[:], adjexp[:], invs[:])

                adjnT = const.tile([P, 4, P], F32)
                for jt in range(4):
                    pt = ptail.tile([P, P], F32, tag="pt")
                    nc.tensor.transpose(
                        pt[:], adjn[:, bass.ts(jt, P)], ident_sb
                    )
                    nc.any.tensor_copy(adjnT[:, jt, :], pt[:])

                aggT_sb = const.tile([P, 2, P], F32)
                for dh in range(2):
                    pa = ptail.tile([P, P], F32, tag="pa")
                    for jt in range(4):
                        nc.tensor.matmul(
                            pa[:],
                            xb_sb(jt)[:, bass.ts(dh, P)],
                            adjnT[:, jt, :],
                            start=(jt == 0),
                            stop=(jt == 3),
                        )
                    nc.any.tensor_copy(aggT_sb[:, dh, :], pa[:])

                po = ptail.tile([P, D], F32, tag="po")
                for dh in range(2):
                    nc.tensor.matmul(
                        po[:], aggT_sb[:, dh, :], wg_sb(dh),
                        start=(dh == 0), stop=(dh == 1),
                    )
                out_sb = const.tile([P, D], F32)
                nc.vector.tensor_add(out_sb[:], po[:], bgt_sb)
                nc.sync.dma_start(out_d[:], out_sb[:])

    nc.compile()
    return nc


_NC = None


def _get_program():
    global _NC
    if _NC is None:
        _NC = _build_program()
    return _NC


def _core_inputs(x, base_adj, W1, b1, W2, b2, W3, b3, Wg, bg, core):
    b, blk = divmod(core, 4)
    i0 = blk * P
    f32 = np.float32
    bf16 = ml_dtypes.bfloat16

    xbf = np.ascontiguousarray(x[b], dtype=f32)               # [512, 256]
    xtf = np.ascontiguousarray(xbf.T)                         # [256, 512]
    w3 = np.asarray(W3, dtype=f32)[:, 0]                      # [128]

    sgnsh = np.zeros((P, 64), dtype=f32)
    sgnsh[:, 32] = np.sign(w3)
    w2s = np.ascontiguousarray(W2.astype(f32) * np.abs(w3)[None, :])

    bha = 0.5 * base_adj[b, i0 : i0 + P, :].astype(f32)
    eye = np.zeros((P, N), dtype=f32)
    eye[np.arange(P), i0 + np.arange(P)] = 1.0

    W1 = np.asarray(W1, f32)

    bigh = np.concatenate(
        [
            xtf[:128, :], xtf[128:, :],                        # xt d-tiles
            W1[D:D + 128, :], W1[D + 128 :, :],                # w1b d-tiles
            w2s[:128, :], w2s[128:, :],                        # w2s c-tiles
            sgnsh,
        ],
        axis=1,
    ).astype(bf16)
    xtif = np.ascontiguousarray(xbf[i0 : i0 + P, :].T)         # [256, 128]
    bigf = np.concatenate(
        [
            xtif[:128, :], xtif[128:, :],                      # xti d-tiles
            W1[:128, :], W1[128:D, :],                         # w1a d-tiles
            np.asarray(b1, f32).reshape(2, P).T,               # b1c [P, 2]
            (np.abs(w3) * np.asarray(b2, f32)).reshape(P, 1),  # b2sc
            np.full((P, 1), 0.5 * float(np.asarray(b3).reshape(-1)[0]), f32),
        ],
        axis=1,
    )
    bigl = np.concatenate(
        [
            xbf[0:128], xbf[128:256], xbf[256:384], xbf[384:],  # xb j-tiles
            np.asarray(Wg, f32)[:128, :], np.asarray(Wg, f32)[128:, :],
            bha, bha + eye,
            np.eye(P, dtype=f32),
            np.tile(np.asarray(bg, f32)[None, :], (P, 1)),
        ],
        axis=1,
    )
    assert bigh.shape[1] == BH_TOT and bigf.shape[1] == BF_TOT
    assert bigl.shape[1] == BL_TOT
    return {
        "bigh": np.ascontiguousarray(bigh),
        "bigf": np.ascontiguousarray(bigf),
        "bigl": np.ascontiguousarray(bigl),
    }


def run(trace=False, **inputs):
    nc = _get_program()
    in_maps = [_core_inputs(core=c, **inputs) for c in range(NCORES)]
    res = run_bass_kernel_spmd(
        nc, in_maps, core_ids=list(range(NCORES)), trace=trace
    )
    out = np.empty((2, N, D), dtype=np.float32)
    for c in range(NCORES):
        b, blk = divmod(c, 4)
        out[b, blk * P : (blk + 1) * P, :] = res.results[c]["out"]
    return out, res


def kernel(**inputs):
    out, _ = run(**inputs)
    return out


# revision 25
# speedup vs baseline: 4923.1032x; 4075.5042x over previous
"""DynamicGCN edge-MLP message passing kernel for 8x trn2 NeuronCores.

Shapes (hardcoded): x [2, 512, 256] f32, base_adj [2, 512, 512] f32,
W1 [512, 256], b1 [256], W2 [256, 128], b2 [128], W3 [128, 1], b3 [1],
Wg [256, 256], bg [256].  Output [2, 512, 256] f32.

Sharding: core c handles batch b = c // 4 and query rows
i in [128*(c%4), 128*(c%4)+128).  Params replicated; all per-core
variation is input data (same NEFF on all 8 cores).

Per core (i-block of 128 query rows, j = all 512 nodes):
  leftb[c,i]  = (x_i @ W1[:D])[c] + b1[c]          (PE prologue)
  rightT[c,j] = (x_j @ W1[D:])[c]                  (PE prologue, bf16)
  per i:
    hT[c,j]   = relu(rightT[c,j] + leftb[c,i])     (DVE dual-scalar op, bf16)
    h2p[k,j]  = sum_c hT[c,j] * (W2*|W3|)[c,k]     (PE, 2 accum matmuls)
    t[k,j]    = relu(h2p[k,j] + |W3_k| b2_k)       (ACT, psum->sbuf, bf16)
    edge[i,j] += sum_k sign(W3_k) t[k,j]           (PE matmul; shifted
                  sign-column stationary operand accumulates psum row i
                  via 32-wide output col-groups)
  edge[i,:] == h2(i,j,:) @ W3  exactly (|W3| folded into W2/b2, sign into
  the reduction weights; relu is positively homogeneous)
  s = tanh(.5*edge + .5*b3)  => sigmoid without a table switch
  adj = .5*badj*(1+s) + eye = .5*badj*s + (.5*badj + eye)
  adjn = softmax(adj) rowwise (exp with fused accum row-sum, reciprocal)
  out = (adjn @ x_b) @ Wg + bg  (PE transposes + 2 matmul chains)

Inputs are packed host-side into 3 DRAM tensors (one bf16 critical, one
f32 critical, one f32 late) so the prologue is 3 large DMAs instead of
26 small ones.
"""

import ml_dtypes
import numpy as np

import concourse.bacc as bacc
import concourse.bass as bass
import concourse.mybir as mybir
import concourse.tile as tile
from concourse.bass_utils import run_bass_kernel_spmd

F32 = mybir.dt.float32
BF16 = mybir.dt.bfloat16
AF = mybir.ActivationFunctionType
OP = mybir.AluOpType

P = 128      # partitions / i-block
N = 512      # nodes (j dim)
D = 256      # input dim
H = 256      # hidden (c dim, 2 partition tiles)
H2 = 128     # second hidden (k dim)
NCORES = 8
LAG = 4      # software-pipeline depth (in 2-row groups)

# --- packed-input layouts (free-dim element offsets) ---
# bigh (bf16): xt[2]@512, w1b[2]@256(c x2 tiles of 256? see below), w2s[2]@128, sgnsh@64
#   xt tiles:   [0:512), [512:1024)         (d-tile rows x j)
#   w1b tiles:  [1024:1280), [1280:1536)    (d-tile rows x c)  (256 each)
#   w2s tiles:  [1536:1664), [1664:1792)    (c-tile rows x k)
#   sgnsh:      [1792:1856)
BH_XT = 0
BH_W1B = 1024
BH_W2S = 1536
BH_SGN = 1792
BH_TOT = 1856
# bigf (f32, critical): xti[2]@128, w1a[2]@256, b1c@2, b2sc@1, halfb3@1
BF_XTI = 0
BF_W1A = 256
BF_B1C = 768
BF_B2SC = 770
BF_HB3 = 771
BF_TOT = 772
# bigl (f32, late): xb[4]@256, wg[2]@256, bha@512, bhae@512, ident@128, bgt@256
BL_XB = 0
BL_WG = 1024
BL_BHA = 1536
BL_BHAE = 2048
BL_ID = 2560
BL_BGT = 2688
BL_TOT = 2944


def _build_program(reps=1):
    """reps>1 wraps the whole kernel body in a For_i loop — used only by
    the timing bench (wall-clock regression over reps)."""
    import contextlib

    nc = bacc.Bacc("TRN2", target_bir_lowering=False, debug=False)

    bigh = nc.dram_tensor("bigh", [P, BH_TOT], BF16, kind="ExternalInput").ap()
    bigf = nc.dram_tensor("bigf", [P, BF_TOT], F32, kind="ExternalInput").ap()
    bigl = nc.dram_tensor("bigl", [P, BL_TOT], F32, kind="ExternalInput").ap()
    out_d = nc.dram_tensor("out", [P, D], F32, kind="ExternalOutput").ap()

    with tile.TileContext(nc) as tc:
        with (
            tc.tile_pool(name="const", bufs=1) as const,
            tc.tile_pool(name="work", bufs=8) as work,
            tc.tile_pool(name="pedge", bufs=1, space="PSUM") as pedge,
        ):
            bh = const.tile([P, BH_TOT], BF16)
            bf = const.tile([P, BF_TOT], F32)
            bl = const.tile([P, BL_TOT], F32)
            loop_cm = tc.For_i(0, reps, 1) if reps > 1 else contextlib.nullcontext()
            loop_cm.__enter__()
            nc.sync.dma_start(bh[:], bigh)
            nc.sync.dma_start(bf[:], bigf)
            nc.gpsimd.dma_start(bl[:], bigl)

            def xt_sb(dt):
                return bh[:, BH_XT + 512 * dt : BH_XT + 512 * (dt + 1)]

            def w1b_sb(dt, csl):
                base = BH_W1B + 256 * dt
                return bh[:, base + csl * 128 : base + csl * 128 + 128]

            def w2s_sb(ct):
                return bh[:, BH_W2S + 128 * ct : BH_W2S + 128 * (ct + 1)]

            sgnsh_sb = bh[:, BH_SGN : BH_SGN + 64]

            def xti_sb(dt):
                return bf[:, BF_XTI + 128 * dt : BF_XTI + 128 * (dt + 1)]

            def w1a_sb(dt, csl):
                base = BF_W1A + 256 * dt
                return bf[:, base + csl * 128 : base + csl * 128 + 128]

            b1c_sb = bf[:, BF_B1C : BF_B1C + 2]
            b2sc_sb = bf[:, BF_B2SC : BF_B2SC + 1]
            halfb3_sb = bf[:, BF_HB3 : BF_HB3 + 1]

            def xb_sb(jt):
                return bl[:, BL_XB + 256 * jt : BL_XB + 256 * (jt + 1)]

            def wg_sb(dt):
                return bl[:, BL_WG + 256 * dt : BL_WG + 256 * (dt + 1)]

            bha_sb = bl[:, BL_BHA : BL_BHA + 512]
            bhae_sb = bl[:, BL_BHAE : BL_BHAE + 512]
            ident_sb = bl[:, BL_ID : BL_ID + 128]
            bgt_sb = bl[:, BL_BGT : BL_BGT + 256]

            # Preload the exp/tanh/relu activation table set early so the
            # ~2.7us table DMA overlaps the input DMAs.
            warm = const.tile([P, 1], F32)
            nc.vector.memset(warm[:], 0.0)
            nc.scalar.activation(warm[:], warm[:], AF.Exp)

            edge_ps = pedge.tile([P, N], F32)

            with tc.tile_pool(name="ph2", bufs=3, space="PSUM") as ph2:
                # ---- prologue: rightT / leftb ----
                rightT_sb = const.tile([P, 2, N], BF16)
                leftb_sb = const.tile([P, 2, P], F32)
                for ct in range(2):
                    ps = ph2.tile([P, 2, N], F32, tag="h2")
                    for dt in range(2):
                        nc.tensor.matmul(
                            ps[:, 0, :],
                            w1b_sb(dt, ct),
                            xt_sb(dt),
                            start=(dt == 0),
                            stop=(dt == 1),
                        )
                    nc.scalar.copy(rightT_sb[:, ct, :], ps[:, 0, :])
                for ct in range(2):
                    ps = ph2.tile([P, 2, N], F32, tag="h2")
                    for dt in range(2):
                        nc.tensor.matmul(
                            ps[:, 0, :P],
                            w1a_sb(dt, ct),
                            xti_sb(dt),
                            start=(dt == 0),
                            stop=(dt == 1),
                        )
                    nc.scalar.activation(
                        leftb_sb[:, ct, :], ps[:, 0, :P], AF.Identity,
                        bias=b1c_sb[:, ct : ct + 1], scale=1.0,
                    )

                # ---- main loop over the 128 query rows, 2 rows per group ----
                NG = P // 2
                h2ps = {}
                for step in range(NG + LAG):
                    if step < NG:
                        g = step
                        hts = []
                        for u in range(2):
                            i = 2 * g + u
                            ht0 = work.tile([P, N], BF16, tag=f"ht0{u}")
                            ht1 = work.tile([P, N], BF16, tag=f"ht1{u}")
                            nc.vector.tensor_scalar(
                                ht0[:], rightT_sb[:, 0, :],
                                leftb_sb[:, 0, i : i + 1], 0.0,
                                op0=OP.add, op1=OP.max,
                            )
                            nc.vector.tensor_scalar(
                                ht1[:], rightT_sb[:, 1, :],
                                leftb_sb[:, 1, i : i + 1], 0.0,
                                op0=OP.add, op1=OP.max,
                            )
                            hts.append((ht0, ht1))
                        ps = ph2.tile([P, 2, N], F32, tag="h2")
                        for u in range(2):
                            nc.tensor.matmul(
                                ps[:, u, :], w2s_sb(0), hts[u][0][:],
                                start=True, stop=False,
                            )
                        for u in range(2):
                            nc.tensor.matmul(
                                ps[:, u, :], w2s_sb(1), hts[u][1][:],
                                start=False, stop=True,
                            )
                        h2ps[g] = ps
                    gj = step - LAG
                    if gj >= 0:
                        t_sb = work.tile([P, 2, N], BF16, tag="tt")
                        nc.scalar.activation(
                            t_sb[:], h2ps.pop(gj)[:], AF.Relu,
                            bias=b2sc_sb[:], scale=1.0,
                        )
                        for u in range(2):
                            j = 2 * gj + u
                            cg, r = divmod(j, 32)
                            nc.tensor.matmul(
                                edge_ps[32 * cg : 32 * cg + 32, :],
                                sgnsh_sb[:, 32 - r : 64 - r],
                                t_sb[:, u, :],
                                start=(r == 0),
                                stop=(r == 31),
                                tile_position=(0, 32 * cg),
                            )

            # ---- tail: sigmoid via tanh, softmax, aggregate, project ----
            with tc.tile_pool(name="ptail", bufs=2, space="PSUM") as ptail:
                s_sb = const.tile([P, N], F32)
                nc.scalar.activation(
                    s_sb[:], edge_ps[:], AF.Tanh, bias=halfb3_sb, scale=0.5
                )
                m1 = const.tile([P, N], F32)
                nc.vector.tensor_mul(m1[:], s_sb[:], bha_sb)
                m2 = const.tile([P, N], F32)
                nc.vector.tensor_add(m2[:], m1[:], bhae_sb)
                adjexp = const.tile([P, N], F32)
                rowsum = const.tile([P, 1], F32)
                nc.scalar.activation(adjexp[:], m2[:], AF.Exp, accum_out=rowsum[:])
                invs = const.tile([P, 1], F32)
                nc.vector.reciprocal(invs[:], rowsum[:])
                adjn = const.tile([P, N], F32)
                nc.vector.tensor_scalar_mul(adjn[:], adjexp[:], invs[:])

                adjnT = const.tile([P, 4, P], F32)
                for jt in range(4):
                    pt = ptail.tile([P, P], F32, tag="pt")
                    nc.tensor.transpose(
                        pt[:], adjn[:, bass.ts(jt, P)], ident_sb
                    )
                    nc.any.tensor_copy(adjnT[:, jt, :], pt[:])

                aggT_sb = const.tile([P, 2, P], F32)
                for dh in range(2):
                    pa = ptail.tile([P, P], F32, tag="pa")
                    for jt in range(4):
                        nc.tensor.matmul(
                            pa[:],
                            xb_sb(jt)[:, bass.ts(dh, P)],
                            adjnT[:, jt, :],
                            start=(jt == 0),
                            stop=(jt == 3),
                        )
                    nc.any.tensor_copy(aggT_sb[:, dh, :], pa[:])

                po = ptail.tile([P, D], F32, tag="po")
                for dh in range(2):
                    nc.tensor.matmul(
                        po[:], aggT_sb[:, dh, :], wg_sb(dh),
                        start=(dh == 0), stop=(dh == 1),
                    )
                out_sb = const.tile([P, D], F32)
                nc.vector.tensor_add(out_sb[:], po[:], bgt_sb)
                nc.sync.dma_start(out_d[:], out_sb[:])
            loop_cm.__exit__(None, None, None)

    nc.compile()
    return nc


_NC = None


def _get_program():
    global _NC
    if _NC is None:
        _NC = _build_program()
    return _NC


def _core_inputs(x, base_adj, W1, b1, W2, b2, W3, b3, Wg, bg, core):
    b, blk = divmod(core, 4)
    i0 = blk * P
    f32 = np.float32
    bf16 = ml_dtypes.bfloat16

    xbf = np.ascontiguousarray(x[b], dtype=f32)               # [512, 256]
    xtf = np.ascontiguousarray(xbf.T)                         # [256, 512]
    w3 = np.asarray(W3, dtype=f32)[:, 0]                      # [128]

    sgnsh = np.zeros((P, 64), dtype=f32)
    sgnsh[:, 32] = np.sign(w3)
    w2s = np.ascontiguousarray(W2.astype(f32) * np.abs(w3)[None, :])

    bha = 0.5 * base_adj[b, i0 : i0 + P, :].astype(f32)
    eye = np.zeros((P, N), dtype=f32)
    eye[np.arange(P), i0 + np.arange(P)] = 1.0

    W1 = np.asarray(W1, f32)

    bigh = np.concatenate(
        [
            xtf[:128, :], xtf[128:, :],                        # xt d-tiles
            W1[D:D + 128, :], W1[D + 128 :, :],                # w1b d-tiles
            w2s[:128, :], w2s[128:, :],                        # w2s c-tiles
            sgnsh,
        ],
        axis=1,
    ).astype(bf16)
    xtif = np.ascontiguousarray(xbf[i0 : i0 + P, :].T)         # [256, 128]
    bigf = np.concatenate(
        [
            xtif[:128, :], xtif[128:, :],                      # xti d-tiles
            W1[:128, :], W1[128:D, :],                         # w1a d-tiles
            np.asarray(b1, f32).reshape(2, P).T,               # b1c [P, 2]
            (np.abs(w3) * np.asarray(b2, f32)).reshape(P, 1),  # b2sc
            np.full((P, 1), 0.5 * float(np.asarray(b3).reshape(-1)[0]), f32),
        ],
        axis=1,
    )
    bigl = np.concatenate(
        [
            xbf[0:128], xbf[128:256], xbf[256:384], xbf[384:],  # xb j-tiles
            np.asarray(Wg, f32)[:128, :], np.asarray(Wg, f32)[128:, :],
            bha, bha + eye,
            np.eye(P, dtype=f32),
            np.tile(np.asarray(bg, f32)[None, :], (P, 1)),
        ],
        axis=1,
    )
    assert bigh.shape[1] == BH_TOT and bigf.shape[1] == BF_TOT
    assert bigl.shape[1] == BL_TOT
    return {
        "bigh": np.ascontiguousarray(bigh),
        "bigf": np.ascontiguousarray(bigf),
        "bigl": np.ascontiguousarray(bigl),
    }


def run(trace=False, **inputs):
    nc = _get_program()
    in_maps = [_core_inputs(core=c, **inputs) for c in range(NCORES)]
    res = run_bass_kernel_spmd(
        nc, in_maps, core_ids=list(range(NCORES)), trace=trace
    )
    out = np.empty((2, N, D), dtype=np.float32)
    for c in range(NCORES):
        b, blk = divmod(c, 4)
        out[b, blk * P : (blk + 1) * P, :] = res.results[c]["out"]
    return out, res


def kernel(**inputs):
    out, _ = run(**inputs)
    return out
